# revision 17
# baseline (speedup 1.0000x reference)
import os
import numpy as np

# nn_Block_SpeGroup — Bass/Tile kernel for 8 NeuronCores (TRN2).
# Data-parallel over batch: 32 samples -> 4 per core, one NEFF run SPMD.
#
# Per-core layout (BS=4 samples):
#   partitions p = (b, h); b in 0..3, h in 0..31
#   xs_all[(b,h), (w, d')], d' = 32*g + j  (channel perm pi: natural d = 4j+g)
#       value = silu(conv_w*xx + conv_b)[b, 4j+g, h, w]
#   direction g reads xs_all via strided/reversed free APs:
#       g0: l=(j,w)   g1: l=(w,j)   g2: rev(g0)   g3: rev(g1)
#   scan per (g, n) via DVE tensor_tensor_scan:  h_t = decay_t*h_{t-1} + dub_t
#       decay = exp(A[g,h,n]*delta) on ACT, dub = delta*u*B on DVE
#   y_g = sum_n h*C via PE identity-matmul PSUM accumulation, + D*xs residual
#   merge: yy[(b,h), wt*128 + 4*w_sp + g] = y_g[(b,h), l_g(j=wt, w_sp)]
#   SE-gate f2 -> LayerNorm over d (ln_g folded into out_proj) -> z-gate -> out_proj.

B, H, W, DIM = 32, 32, 32, 128
K, N, R = 4, 16, 2
DS = W
EPS = 1e-5
NCORES = 8
BS = B // NCORES
L = 1024
P = 128

LAST_HW_EXEC_NS = None

_BASS_CACHE = {}


def _prepare_consts(in_proj_w, conv_w, conv_b, fc1_w, fc1_b, fc2_w, fc2_b,
                    x_proj_weight, dt_projs_weight, dt_projs_bias, A_logs, Ds,
                    ln_g, ln_b, out_proj_w):
    """Host-side weight fusion/permutation (tiny O(params) work)."""
    f32 = np.float32
    dp = np.arange(128)
    p2n = 4 * (dp % 32) + dp // 32          # d' -> natural d

    c = {}
    rhs_xz = np.empty((128, 256), f32)
    rhs_xz[:, :128] = (in_proj_w[p2n, :] * conv_w[p2n, None]).T
    rhs_xz[:, 128:] = in_proj_w[128:, :].T
    c['rhs_xz'] = rhs_xz
    bias_row = np.zeros((1, 256), f32)
    bias_row[0, :128] = conv_b[p2n]
    c['bias_row'] = bias_row

    # x_dbl path (contract over scan-row h = DS dim); shipped partition-major.
    lhsT_bc = np.zeros((K, 128, 128), f32)   # [g, (b,h), (b,c)]
    lhsT_dt = np.zeros((K, 128, 4 * R), f32)
    lhsT_delta = np.zeros((K, 4 * R, 128), f32)
    for g in range(K):
        blk_bc = x_proj_weight[g, R:, :].T          # (h, c=32)
        blk_dt = x_proj_weight[g, :R, :].T          # (h, r)
        blk_dl = dt_projs_weight[g, :, :].T         # (r, h)
        for b in range(BS):
            lhsT_bc[g, 32 * b:32 * b + 32, 32 * b:32 * b + 32] = blk_bc
            lhsT_dt[g, 32 * b:32 * b + 32, R * b:R * b + R] = blk_dt
            lhsT_delta[g, R * b:R * b + R, 32 * b:32 * b + 32] = blk_dl
    c['lhsT_bc'] = np.ascontiguousarray(lhsT_bc.transpose(1, 0, 2))      # (128, K, 128)
    c['lhsT_dt'] = np.ascontiguousarray(lhsT_dt.transpose(1, 0, 2))      # (128, K, 8)
    c['lhsT_delta'] = np.ascontiguousarray(lhsT_delta.transpose(1, 0, 2))  # (8, K, 128)

    dtb = dt_projs_bias.reshape(K, DS)
    c['dtb_rep'] = np.tile(dtb.T, (4, 1)).astype(f32)                    # (128, K)
    A = -np.exp(A_logs.astype(np.float64)).reshape(K, DS, N).astype(f32)
    c['a_rep'] = np.tile(A.transpose(1, 0, 2).reshape(DS, K * N), (4, 1)).astype(f32)  # (128, 64)

    Dkh = Ds.reshape(K, DS)
    ddiag = np.zeros((K, 128, 128), f32)
    for g in range(K):
        ddiag[g] = np.diag(np.tile(Dkh[g], 4))
    c['ddiag'] = np.ascontiguousarray(ddiag.transpose(1, 0, 2))          # (128, K, 128)

    E = np.zeros((2 * N, 128, 128), f32)     # [(2n+e), (b,c), (b,h)]
    for n in range(N):
        for e in range(2):
            for b in range(BS):
                E[2 * n + e, 32 * b + n + 16 * e, 32 * b:32 * b + 32] = 1.0
    c['e_all'] = np.ascontiguousarray(E.transpose(1, 0, 2))              # (128, 32, 128)

    bones = np.zeros((128, BS), f32)
    for b in range(BS):
        bones[32 * b:32 * b + 32, b] = 1.0 / (H * W)
    c['blockones'] = bones
    c['fc1T'] = np.ascontiguousarray(fc1_w.T[p2n, :]).astype(f32)        # (d'pi, 4)
    c['fc1b'] = fc1_b.reshape(4, 1).astype(f32)
    c['fc2T'] = np.ascontiguousarray(fc2_w.T).astype(f32)                # (4, d natural)
    c['fc2b_row'] = fc2_b.reshape(1, 128).astype(f32)
    bones4 = np.zeros((BS, 128), f32)
    for b in range(BS):
        bones4[b, 32 * b:32 * b + 32] = 1.0
    c['bones4'] = bones4

    c['ident_r'] = np.eye(128, dtype=f32)
    c['wout_g'] = np.ascontiguousarray(out_proj_w.T * ln_g[:, None]).astype(f32)
    c['wout_b'] = np.ascontiguousarray(out_proj_w.T * ln_b[:, None]).astype(f32)
    c['ln_b'] = ln_b.astype(f32)
    return c


def _build_bass(use_b_term):
    key = ('v1', use_b_term)
    if key in _BASS_CACHE:
        return _BASS_CACHE[key]

    import concourse.bass as bass
    import concourse.bacc as bacc
    import concourse.mybir as mybir
    import concourse.tile as tile
    from concourse.masks import make_identity
    from contextlib import ExitStack

    f32 = mybir.dt.float32
    f32r = mybir.dt.float32r
    AF = mybir.ActivationFunctionType
    OP = mybir.AluOpType

    nc = bacc.Bacc("TRN2", target_bir_lowering=False)

    x_d = nc.dram_tensor("x", [BS, H, W, DIM], f32, kind="ExternalInput")
    out_d = nc.dram_tensor("out", [BS, H, W, DIM], f32, kind="ExternalOutput")
    dt_in = {}
    F32R_INPUTS = {"e_all", "bones4", "ident_r"}
    for name, shape in [
        ("rhs_xz", [128, 256]), ("bias_row", [1, 256]),
        ("lhsT_bc", [128, K, 128]), ("lhsT_dt", [128, K, 4 * R]),
        ("lhsT_delta", [4 * R, K, 128]), ("dtb_rep", [128, K]),
        ("a_rep", [128, K * N]), ("ddiag", [128, K, 128]),
        ("e_all", [128, 2 * N, 128]), ("blockones", [128, BS]),
        ("ident_r", [128, 128]),
        ("fc1T", [128, 4]), ("fc1b", [4, 1]), ("fc2T", [4, 128]),
        ("fc2b_row", [1, 128]), ("bones4", [BS, 128]),
        ("wout_g", [128, 128]), ("wout_b", [128, 128]),
    ]:
        dt_in[name] = nc.dram_tensor(
            name, shape, f32r if name in F32R_INPUTS else f32,
            kind="ExternalInput")

    with tile.TileContext(nc) as tc, ExitStack() as ctx:
        consts = ctx.enter_context(tc.tile_pool(name="consts", bufs=1))
        xin = ctx.enter_context(tc.tile_pool(name="xin", bufs=3))
        xt_sb = ctx.enter_context(tc.tile_pool(name="xt_sb", bufs=3))
        big = ctx.enter_context(tc.tile_pool(name="big", bufs=1))
        gwork = ctx.enter_context(tc.tile_pool(name="gwork", bufs=2))
        nwork = ctx.enter_context(tc.tile_pool(name="nwork", bufs=2))
        small = ctx.enter_context(tc.tile_pool(name="small", bufs=2))
        # PSUM: 8 banks total (16KB/partition). Four 2-bank pools.
        ps_y = ctx.enter_context(tc.tile_pool(name="ps_y", bufs=1, space="PSUM"))
        ps_b = ctx.enter_context(tc.tile_pool(name="ps_b", bufs=1, space="PSUM"))
        ps_c = ctx.enter_context(tc.tile_pool(name="ps_c", bufs=1, space="PSUM"))
        ps_t = ctx.enter_context(tc.tile_pool(name="ps_t", bufs=1, space="PSUM"))

        ident = consts.tile([P, P], f32)
        make_identity(nc, ident)
        ones1x128 = consts.tile([1, 128], f32)
        nc.vector.memset(ones1x128[:], 1.0)
        ones1x4 = consts.tile([1, 4], f32)
        nc.vector.memset(ones1x4[:], 1.0)
        eps_t = consts.tile([P, 1], f32)
        nc.vector.memset(eps_t[:], EPS)

        cs = {}
        for name, t in dt_in.items():
            shape = list(t.shape)
            tl = consts.tile(shape, f32r if name in F32R_INPUTS else f32,
                             tag=f"c_{name}")
            nc.sync.dma_start(out=tl[:], in_=t[tuple(slice(None) for _ in shape)])
            cs[name] = tl

        xs_all = big.tile([P, W, 128], f32)
        z_all = big.tile([P, W, 128], f32)

        # ---------- phase 1: in_proj + conv + silu ----------
        for w4 in range(0, W, 4):
            ip = ps_t.tile([P, 4, 256], f32, tag="pst")
            for wi in range(4):
                w = w4 + wi
                xt_in = xin.tile([P, 128], f32)
                src = bass.AP(tensor=x_d[:, :, :, :].tensor, offset=w * DIM,
                              ap=[[H * W * DIM, BS], [W * DIM, H], [1, DIM]])
                nc.sync.dma_start(out=xt_in[:], in_=src)
                tp = ps_b.tile([P, 128], f32, tag="psb")
                nc.tensor.transpose(tp[:], xt_in[:], ident[:])
                xT = xt_sb.tile([P, 128], f32, tag="xT")
                nc.scalar.copy(xT[:], tp[:])
                nc.tensor.matmul(ip[:, wi, :], xT[:], cs['rhs_xz'][:],
                                 start=True, stop=False)
                nc.tensor.matmul(ip[:, wi, :], ones1x128[:], cs['bias_row'][:],
                                 start=False, stop=True)
            nc.scalar.activation(xs_all[:, w4:w4 + 4, :], ip[:, :, 0:128], AF.Silu)
            nc.scalar.activation(z_all[:, w4:w4 + 4, :], ip[:, :, 128:256], AF.Silu)

        # ---------- phase 2: selective scan ----------
        xs_base = xs_all[:]
        pdim = xs_base.ap[0]

        def xs_ap(g, half=None, dt=None):
            if g == 0:
                off, dims = 32 * g, [[1, 32], [128, 32]]
            elif g == 1:
                off, dims = 32 * g, [[128, 32], [1, 32]]
            elif g == 2:
                off, dims = 32 * g + 31 + 31 * 128, [[-1, 32], [-128, 32]]
            else:
                off, dims = 32 * g + 31 + 31 * 128, [[-128, 32], [-1, 32]]
            if half is not None:
                dims = [list(dims[0]), list(dims[1])]
                off += half * 16 * dims[0][0]
                dims[0][1] = 16
            ap = bass.AP(tensor=xs_base.tensor, offset=xs_base.offset + off,
                         ap=[pdim] + dims)
            return ap.bitcast(dt) if dt is not None else ap

        yy = big.tile([P, W, 128], f32)

        for g in range(K):
            bcp = ps_t.tile([P, L], f32, tag="pst")
            for hf in range(2):
                nc.tensor.matmul(bcp[:, 512 * hf:512 * (hf + 1)],
                                 cs['lhsT_bc'][:, g, :], xs_ap(g, hf),
                                 start=True, stop=True)
            bc_sb = gwork.tile([P, L], f32r, tag="bc_sb")
            nc.scalar.copy(bc_sb[:], bcp[:])

            dtsp = ps_t.tile([4 * R, L], f32, tag="pst")
            for hf in range(2):
                nc.tensor.matmul(dtsp[:, 512 * hf:512 * (hf + 1)],
                                 cs['lhsT_dt'][:, g, :], xs_ap(g, hf),
                                 start=True, stop=True)
            dts_sb = gwork.tile([4 * R, L], f32, tag="dts_sb")
            nc.scalar.copy(dts_sb[:], dtsp[:])

            dltp = ps_t.tile([P, L], f32, tag="pst")
            for hf in range(2):
                nc.tensor.matmul(dltp[:, 512 * hf:512 * (hf + 1)],
                                 cs['lhsT_delta'][:, g, :],
                                 dts_sb[:, 512 * hf:512 * (hf + 1)],
                                 start=True, stop=True)
            esp = gwork.tile([P, L], f32, tag="esp")
            nc.scalar.activation(esp[:], dltp[:], AF.Exp,
                                 bias=cs['dtb_rep'][:, g:g + 1])
            delta = gwork.tile([P, L], f32, tag="delta")
            nc.scalar.activation(delta[:], esp[:], AF.Ln, bias=1.0)

            du = gwork.tile([P, 32, 32], f32, tag="du")
            nc.vector.tensor_tensor(
                du[:], delta[:].rearrange("p (a b) -> p a b", a=32),
                xs_ap(g), OP.mult)

            yp = ps_y.tile([P, L], f32, tag="psy")
            for n in range(N):
                Bp = ps_b.tile([P, L], f32, tag="psb")
                Cp = ps_c.tile([P, L], f32, tag="psc")
                for hf in range(2):
                    sl = slice(512 * hf, 512 * (hf + 1))
                    nc.tensor.matmul(Bp[:, sl], cs['e_all'][:, 2 * n, :],
                                     bc_sb[:, sl], start=True, stop=True)
                    nc.tensor.matmul(Cp[:, sl], cs['e_all'][:, 2 * n + 1, :],
                                     bc_sb[:, sl], start=True, stop=True)

                decay = nwork.tile([P, L], f32, tag="decay")
                nc.scalar.activation(decay[:], delta[:], AF.Exp,
                                     scale=cs['a_rep'][:, 16 * g + n:16 * g + n + 1])
                dub = nwork.tile([P, L], f32, tag="dub")
                nc.vector.tensor_tensor(dub[:], du[:].rearrange("p a b -> p (a b)"),
                                        Bp[:], OP.mult)
                hsc = nwork.tile([P, L], f32, tag="hsc")
                nc.vector.tensor_tensor_scan(hsc[:], decay[:], dub[:], 0.0,
                                             OP.mult, OP.add)
                hc = nwork.tile([P, L], f32r, tag="hc")
                nc.vector.tensor_tensor(hc[:], hsc[:], Cp[:], OP.mult)
                for hf in range(2):
                    sl = slice(512 * hf, 512 * (hf + 1))
                    nc.tensor.matmul(yp[:, sl], cs['ident_r'][:], hc[:, sl],
                                     start=(n == 0), stop=False)
            for hf in range(2):
                nc.tensor.matmul(yp[:, 512 * hf:512 * (hf + 1)],
                                 cs['ddiag'][:, g, :],
                                 xs_ap(g, hf), start=False, stop=True)

            # un-permute into yy (all dst positive, reversed reads for g2/g3)
            ydim = yy[:].ap[0]
            if g == 0:
                doff, ddims = g, [[128, 32], [4, 32]]
                soff, sdims = 0, [[32, 32], [1, 32]]
            elif g == 1:
                doff, ddims = g, [[4, 32], [128, 32]]
                soff, sdims = 0, [[32, 32], [1, 32]]
            elif g == 2:
                doff, ddims = g, [[128, 32], [4, 32]]
                soff, sdims = 1023, [[-32, 32], [-1, 32]]
            else:
                doff, ddims = g, [[128, 32], [4, 32]]
                soff, sdims = 1023, [[-1, 32], [-32, 32]]
            dst = bass.AP(tensor=yy[:].tensor, offset=yy[:].offset + doff,
                          ap=[ydim] + ddims)
            srcp = yp[:]
            src = bass.AP(tensor=srcp.tensor, offset=srcp.offset + soff,
                          ap=[srcp.ap[0]] + sdims)
            nc.scalar.copy(dst, src)

        # ---------- phase 3: SE gate ----------
        red = small.tile([P, 128], f32, tag="red")
        nc.vector.tensor_reduce(red[:], xs_all[:].rearrange("p w d -> p d w"),
                                mybir.AxisListType.X, OP.add)
        zzp = ps_t.tile([BS, 128], f32, tag="pst")
        nc.tensor.matmul(zzp[:], cs['blockones'][:], red[:], start=True, stop=True)
        zz_sb = small.tile([BS, 128], f32, tag="zz_sb")
        nc.scalar.copy(zz_sb[:], zzp[:])
        zzTp = ps_t.tile([P, BS], f32, tag="pst")
        nc.tensor.transpose(zzTp[:], zz_sb[:], ident[0:BS, 0:BS])
        zzT = small.tile([P, BS], f32, tag="zzT")
        nc.scalar.copy(zzT[:], zzTp[:])
        f1p = ps_t.tile([4, BS], f32, tag="pst")
        nc.tensor.matmul(f1p[:], cs['fc1T'][:], zzT[:], start=True, stop=True)
        f1 = small.tile([4, BS], f32, tag="f1")
        nc.scalar.activation(f1[:], f1p[:], AF.Relu, bias=cs['fc1b'][:])
        f2p = ps_t.tile([BS, 128], f32, tag="pst")
        nc.tensor.matmul(f2p[:], f1[:], cs['fc2T'][:], start=True, stop=False)
        nc.tensor.matmul(f2p[:], ones1x4[:], cs['fc2b_row'][:], start=False, stop=True)
        sg1 = small.tile([BS, 128], f32, tag="sg1")
        nc.scalar.activation(sg1[:], f2p[:], AF.Exp, scale=-1.0)
        sg2 = small.tile([BS, 128], f32, tag="sg2")
        nc.scalar.activation(sg2[:], sg1[:], AF.Ln, bias=1.0)
        f2row = small.tile([BS, 128], f32r, tag="f2row")
        nc.scalar.activation(f2row[:], sg2[:], AF.Exp, scale=-1.0)
        f2wide = big.tile([BS, W, 128], f32r, tag="f2wide")
        f2src = bass.AP(tensor=f2row[:].tensor, offset=f2row[:].offset,
                        ap=[f2row[:].ap[0], [0, W], [1, 128]])
        nc.sync.dma_start(out=f2wide[:], in_=f2src)
        for q in range(8):
            f2bc = ps_t.tile([P, 4, 128], f32, tag="pst")
            nc.tensor.matmul(
                f2bc[:].rearrange("p a b -> p (a b)"),
                cs['bones4'][:],
                f2wide[:, 4 * q:4 * q + 4, :].rearrange("p a b -> p (a b)"),
                start=True, stop=True)
            nc.vector.tensor_tensor(yy[:, 4 * q:4 * q + 4, :],
                                    yy[:, 4 * q:4 * q + 4, :], f2bc[:], OP.mult)

        # ---------- phase 4: LayerNorm + z gate ----------
        sq = big.tile([P, W, 128], f32)
        nc.scalar.square(sq[:], yy[:])
        mu = small.tile([P, W], f32, tag="mu")
        msq = small.tile([P, W], f32, tag="msq")
        nc.vector.tensor_reduce(mu[:], yy[:], mybir.AxisListType.X, OP.add)
        nc.vector.tensor_reduce(msq[:], sq[:], mybir.AxisListType.X, OP.add)
        nc.vector.tensor_scalar(mu[:], mu[:], 1.0 / 128, None, OP.mult)
        nc.vector.tensor_scalar(msq[:], msq[:], 1.0 / 128, None, OP.mult)
        mu2 = small.tile([P, W], f32, tag="mu2")
        nc.vector.tensor_tensor(mu2[:], mu[:], mu[:], OP.mult)
        var = small.tile([P, W], f32, tag="var")
        nc.vector.tensor_tensor(var[:], msq[:], mu2[:], OP.subtract)
        lnv = small.tile([P, W], f32, tag="lnv")
        nc.scalar.activation(lnv[:], var[:], AF.Ln, bias=eps_t[:])
        rstd = small.tile([P, W], f32, tag="rstd")
        nc.scalar.activation(rstd[:], lnv[:], AF.Exp, scale=-0.5)
        for w in range(W):
            nc.vector.tensor_scalar(yy[:, w, :], yy[:, w, :],
                                    mu[:, w:w + 1], rstd[:, w:w + 1],
                                    OP.subtract, OP.mult)
        nc.vector.tensor_tensor(yy[:], yy[:], z_all[:], OP.mult)

        # ---------- phase 5: out_proj ----------
        for w in range(W):
            tp = ps_b.tile([P, 128], f32, tag="psb")
            nc.tensor.transpose(tp[:], yy[:, w, :], ident[:])
            tz = xt_sb.tile([P, 128], f32, tag="tz")
            nc.scalar.copy(tz[:], tp[:])
            op = ps_c.tile([P, 128], f32, tag="psc")
            nc.tensor.matmul(op[:], cs['wout_g'][:], tz[:], start=True,
                             stop=not use_b_term)
            if use_b_term:
                tpz = ps_b.tile([P, 128], f32, tag="psb")
                nc.tensor.transpose(tpz[:], z_all[:, w, :], ident[:])
                tzz = xt_sb.tile([P, 128], f32, tag="tzz")
                nc.scalar.copy(tzz[:], tpz[:])
                nc.tensor.matmul(op[:], cs['wout_b'][:], tzz[:], start=False,
                                 stop=True)
            osb = xt_sb.tile([P, 128], f32, tag="osb")
            nc.scalar.copy(osb[:], op[:])
            dst = bass.AP(tensor=out_d[:, :, :, :].tensor, offset=w * DIM,
                          ap=[[1, DIM], [H * W * DIM, BS], [W * DIM, H]])
            nc.sync.dma_start(out=dst,
                              in_=osb[:].rearrange("p (a b) -> p a b", a=BS))

    nc.compile()
    _BASS_CACHE[key] = nc
    return nc


def kernel(**inputs):
    global LAST_HW_EXEC_NS
    inputs = {k: np.ascontiguousarray(np.asarray(v, dtype=np.float32))
              for k, v in inputs.items()}
    x = inputs.pop('x')
    c = _prepare_consts(**inputs)
    use_b_term = bool(np.any(c['ln_b'] != 0.0))
    nc = _build_bass(use_b_term)

    from concourse.bass_utils import run_bass_kernel_spmd

    const_map = {k: np.ascontiguousarray(v) for k, v in c.items() if k != 'ln_b'}
    in_maps = []
    for core in range(NCORES):
        m = dict(const_map)
        m['x'] = np.ascontiguousarray(x[core * BS:(core + 1) * BS])
        in_maps.append(m)

    res = run_bass_kernel_spmd(nc, in_maps, core_ids=list(range(NCORES)),
                               trace=False)
    if res.exec_time_ns is not None:
        LAST_HW_EXEC_NS = int(res.exec_time_ns)
    out = np.concatenate([r['out'] for r in res.results], axis=0)
    return out.astype(np.float32)
